# revision 7
# baseline (speedup 1.0000x reference)
"""Trainium2 Bass kernel for nn_DynamycMoE (dense-masked top-2 MoE).

Data-parallel over the 8192-token batch: each of the 8 NeuronCores gets
1024 tokens and a full replica of the (small) expert weights, computes
gating + all-expert MLPs + gated combine fully on device, and returns its
[1024, 512] slice of the output.

Device algorithm per core (feature-major activations, tokens on the free
axis so every matmul streams >=256 columns):
  logitsT     -> top-2 softmax gates (token-major chunks of 128)
  per expert: hT = relu(W1t.T @ xT + b1)   [256 x 512]
              oT = W2t.T @ hT              [64  x 512]
              ogT = (oT + b2) * gate_bcast [64  x 512]
  expert pairs stacked on partitions (2x64 = 128 = K) so the mapper matmul
  accumulates the gated combine directly in PSUM:
              y[tok, 512] += og_pair.T @ Wm_pair  over 4 pairs
  eps-substitution (y==0 -> float64 eps) fused into the PSUM->SBUF copy.
"""

import numpy as np

import concourse.bacc as bacc
import concourse.bass as bass
import concourse.mybir as mybir
import concourse.tile as tile
from concourse import bass_utils

F32 = mybir.dt.float32
F32R = mybir.dt.float32r
AF = mybir.ActivationFunctionType
ALU = mybir.AluOpType

B, D, H, E, C, T = 8192, 768, 256, 8, 64, 512
NCORES = 8
BL = B // NCORES  # tokens per core
TT = 512          # token tile (free-dim) size
NT = BL // TT     # token tiles per core
DC = D // 128     # K-chunks over D
HC = H // 128     # K-chunks over H
NPAIR = E // 2
EPS = float(np.finfo(np.float64).eps)

USE_FP32R = False  # float32r = TF32: 4x faster matmul, 10-bit mantissa


def _r(ap):
    return ap.bitcast(F32R) if USE_FP32R else ap


def _build_nc():
    nc = bacc.Bacc(
        "TRN2", target_bir_lowering=False, debug=False, enable_asserts=False
    )

    # All inputs are pre-arranged on the host into their exact SBUF image
    # [128, free] so every load is one fully-contiguous DMA.
    xT_h = nc.dram_tensor("xT", [128, DC * BL], F32, kind="ExternalInput")
    wg_h = nc.dram_tensor("wg", [128, DC * E], F32, kind="ExternalInput")
    w1_h = nc.dram_tensor("w1", [128, E * DC * H], F32, kind="ExternalInput")
    b1_h = nc.dram_tensor("b1", [128, E * HC], F32, kind="ExternalInput")
    w2_h = nc.dram_tensor("w2", [128, E * HC * C], F32, kind="ExternalInput")
    b2_h = nc.dram_tensor("b2", [64, E], F32, kind="ExternalInput")
    wm_h = nc.dram_tensor("wm", [128, NPAIR * T], F32, kind="ExternalInput")
    id_h = nc.dram_tensor("ident", [128, 128], F32, kind="ExternalInput")
    y_h = nc.dram_tensor("y", [BL, T], F32, kind="ExternalOutput")

    with tile.TileContext(nc) as tc:
        with (
            tc.tile_pool(name="weights", bufs=1) as wpool,
            tc.tile_pool(name="gates", bufs=1) as gpool,
            tc.tile_pool(name="gtmp", bufs=2) as gtmp,
            tc.tile_pool(name="hsb", bufs=2) as hpool,
            tc.tile_pool(name="og", bufs=2) as ogpool,
            tc.tile_pool(name="gb", bufs=4) as gbpool,
            tc.tile_pool(name="yout", bufs=4) as ypool,
        ):
            # ---- resident loads ----
            xT = wpool.tile([128, DC, BL], F32, tag="xT")
            nc.sync.dma_start(xT[:], xT_h[:].rearrange("p (c t) -> p c t", c=DC))
            wg = wpool.tile([128, DC, E], F32, tag="wg")
            nc.sync.dma_start(wg[:], wg_h[:].rearrange("p (c e) -> p c e", c=DC))
            w1 = wpool.tile([128, E, DC, H], F32, tag="w1")
            nc.sync.dma_start(
                w1[:], w1_h[:].rearrange("p (e c h) -> p e c h", e=E, c=DC)
            )
            b1 = wpool.tile([128, E, HC], F32, tag="b1")
            nc.sync.dma_start(b1[:], b1_h[:].rearrange("p (e c) -> p e c", e=E))
            w2 = wpool.tile([128, E, HC, C], F32, tag="w2")
            nc.sync.dma_start(
                w2[:], w2_h[:].rearrange("p (e c k) -> p e c k", e=E, c=HC)
            )
            b2 = wpool.tile([64, E], F32, tag="b2")
            nc.sync.dma_start(b2[:], b2_h[:])
            wm = wpool.tile([128, NPAIR, T], F32, tag="wm")
            nc.sync.dma_start(wm[:], wm_h[:].rearrange("p (g t) -> p g t", g=NPAIR))
            ident = wpool.tile([128, 128], F32, tag="ident")
            nc.sync.dma_start(ident[:], id_h[:])

            gatesT = gpool.tile([8, BL], F32, tag="gatesT")
            # per-expert gate rows flattened into partition 0 (engine ops may
            # only address SBUF partition starts 0/32/64/96, so row e of
            # gatesT is not directly readable; a SBUF->SBUF DMA regathers it)
            gflat = gpool.tile([1, E, BL], F32, tag="gflat")

            # ---- phase A: gating for all BL tokens (token-major chunks) ----
            with tc.tile_pool(
                name="ps_gate", bufs=2, space=bass.MemorySpace.PSUM
            ) as ps_g:
                for q in range(BL // 128):
                    tok = q * 128
                    lg = ps_g.tile([128, E], F32, tag="lg")
                    for kc in range(DC):
                        nc.tensor.matmul(
                            lg[:],
                            xT[:, kc, tok : tok + 128],
                            wg[:, kc, :],
                            start=(kc == 0),
                            stop=(kc == DC - 1),
                        )
                    mx1 = gtmp.tile([128, 1], F32, tag="mx1")
                    nc.vector.reduce_max(mx1[:], lg[:], axis=mybir.AxisListType.X)
                    is1 = gtmp.tile([128, E], F32, tag="is1")
                    nc.vector.tensor_scalar(
                        is1[:], lg[:], mx1[:], None, op0=ALU.is_equal
                    )
                    masked = gtmp.tile([128, E], F32, tag="masked")
                    nc.vector.scalar_tensor_tensor(
                        masked[:], is1[:], -1e30, lg[:], op0=ALU.mult, op1=ALU.add
                    )
                    mx2 = gtmp.tile([128, 1], F32, tag="mx2")
                    nc.vector.reduce_max(mx2[:], masked[:], axis=mybir.AxisListType.X)
                    is2 = gtmp.tile([128, E], F32, tag="is2")
                    nc.vector.tensor_scalar(
                        is2[:], masked[:], mx2[:], None, op0=ALU.is_equal
                    )
                    d = gtmp.tile([128, 1], F32, tag="d")
                    nc.vector.tensor_sub(d[:], mx2[:], mx1[:])
                    ed = gtmp.tile([128, 1], F32, tag="ed")
                    nc.scalar.activation(ed[:], d[:], AF.Exp)
                    den = gtmp.tile([128, 1], F32, tag="den")
                    nc.vector.tensor_scalar_add(den[:], ed[:], 1.0)
                    g1 = gtmp.tile([128, 1], F32, tag="g1")
                    nc.vector.reciprocal(g1[:], den[:])
                    g2 = gtmp.tile([128, 1], F32, tag="g2")
                    nc.vector.tensor_mul(g2[:], ed[:], g1[:])
                    t2 = gtmp.tile([128, E], F32, tag="t2")
                    nc.vector.tensor_scalar_mul(t2[:], is2[:], g2[:])
                    gq = gtmp.tile([128, E], F32, tag="gq")
                    nc.vector.scalar_tensor_tensor(
                        gq[:], is1[:], g1[:], t2[:], op0=ALU.mult, op1=ALU.add
                    )
                    tr = ps_g.tile([8, 128], F32, tag="tr")
                    nc.tensor.transpose(tr[:], gq[:], ident[:])
                    nc.vector.tensor_copy(gatesT[:, tok : tok + 128], tr[:])
                for ti in range(NT):
                    tok0 = ti * TT
                    nc.sync.dma_start(
                        gflat[0:1, :, tok0 : tok0 + TT],
                        gatesT[:, tok0 : tok0 + TT],
                    )

            # ---- phase B/C: experts + combine, one token tile at a time ----
            with (
                tc.tile_pool(name="ps_h", bufs=1, space=bass.MemorySpace.PSUM) as ps_h,
                tc.tile_pool(name="ps_o", bufs=2, space=bass.MemorySpace.PSUM) as ps_o,
                tc.tile_pool(name="ps_y", bufs=1, space=bass.MemorySpace.PSUM) as ps_y,
            ):
                for ti in range(NT):
                    tok0 = ti * TT
                    y_ps = ps_y.tile([128, 4, TT], F32, tag="y")
                    for pair in range(NPAIR):
                        og = ogpool.tile([128, TT], F32, tag="og")
                        for j in range(2):
                            e = 2 * pair + j
                            hT = ps_h.tile([128, HC, TT], F32, tag="h")
                            for half in range(HC):
                                for kc in range(DC):
                                    nc.tensor.matmul(
                                        hT[:, half, :],
                                        _r(w1[:, e, kc, half * 128 : half * 128 + 128]),
                                        _r(xT[:, kc, tok0 : tok0 + TT]),
                                        start=(kc == 0),
                                        stop=(kc == DC - 1),
                                    )
                            hs = hpool.tile([128, HC, TT], F32, tag="hs")
                            for half in range(HC):
                                nc.scalar.activation(
                                    hs[:, half, :],
                                    hT[:, half, :],
                                    AF.Relu,
                                    bias=b1[:, e, half : half + 1],
                                )
                            oT = ps_o.tile([64, TT], F32, tag="o")
                            for kc in range(HC):
                                nc.tensor.matmul(
                                    oT[:],
                                    _r(w2[:, e, kc, :]),
                                    _r(hs[:, kc, :]),
                                    start=(kc == 0),
                                    stop=(kc == HC - 1),
                                )
                            # og[j] = (oT + b2[e]) * gate_e  (gate bcast over C)
                            gb = gbpool.tile([64, TT], F32, tag="gb")
                            nc.gpsimd.partition_broadcast(
                                gb[:], gflat[0:1, e, tok0 : tok0 + TT]
                            )
                            nc.vector.scalar_tensor_tensor(
                                og[j * 64 : j * 64 + 64, :],
                                oT[:],
                                b2[:, e : e + 1],
                                gb[:],
                                op0=ALU.add,
                                op1=ALU.mult,
                            )
                        for q in range(TT // 128):
                            nc.tensor.matmul(
                                y_ps[:, q, :],
                                _r(og[:, q * 128 : q * 128 + 128]),
                                _r(wm[:, pair, :]),
                                start=(pair == 0),
                                stop=(pair == NPAIR - 1),
                            )
                    for q in range(TT // 128):
                        tok = tok0 + q * 128
                        mask = ypool.tile([128, T], F32, tag="mask")
                        nc.vector.tensor_scalar(
                            mask[:], y_ps[:, q, :], 0.0, None, op0=ALU.is_equal
                        )
                        ysb = ypool.tile([128, T], F32, tag="ysb")
                        nc.vector.scalar_tensor_tensor(
                            ysb[:],
                            mask[:],
                            EPS,
                            y_ps[:, q, :],
                            op0=ALU.mult,
                            op1=ALU.add,
                        )
                        nc.sync.dma_start(y_h[tok : tok + 128, :], ysb[:])

    nc.compile()
    return nc


_NC_CACHE = {}


def _get_nc():
    if "nc" not in _NC_CACHE:
        _NC_CACHE["nc"] = _build_nc()
    return _NC_CACHE["nc"]


def _host_prep(x, w_gate, W1, b1, W2, b2, Wm):
    """Rearrange weights into SBUF images; shard + transpose x per core."""
    f = np.float32
    xs = []
    for c in range(NCORES):
        s = np.ascontiguousarray(x[c * BL : (c + 1) * BL].T)  # [D, BL]
        xs.append(
            np.ascontiguousarray(
                s.reshape(DC, 128, BL).transpose(1, 0, 2).reshape(128, DC * BL)
            )
        )
    W1t = W1.transpose(0, 2, 1)  # [E, D, H]
    w1_img = np.ascontiguousarray(
        W1t.reshape(E, DC, 128, H).transpose(2, 0, 1, 3).reshape(128, E * DC * H)
    )
    W2t = W2.transpose(0, 2, 1)  # [E, H, C]
    w2_img = np.ascontiguousarray(
        W2t.reshape(E, HC, 128, C).transpose(2, 0, 1, 3).reshape(128, E * HC * C)
    )
    WmT = Wm.transpose(0, 2, 1)  # [E, C, T]
    wm_img = np.ascontiguousarray(
        WmT.reshape(NPAIR, 128, T).transpose(1, 0, 2).reshape(128, NPAIR * T)
    )
    wg_img = np.ascontiguousarray(
        w_gate.reshape(DC, 128, E).transpose(1, 0, 2).reshape(128, DC * E)
    )
    b1_img = np.ascontiguousarray(
        b1.reshape(E, HC, 128).transpose(2, 0, 1).reshape(128, E * HC)
    )
    b2_img = np.ascontiguousarray(b2.T)  # [C, E]
    ident = np.eye(128, dtype=f)
    shared = {
        "wg": wg_img.astype(f, copy=False),
        "w1": w1_img.astype(f, copy=False),
        "b1": b1_img.astype(f, copy=False),
        "w2": w2_img.astype(f, copy=False),
        "b2": b2_img.astype(f, copy=False),
        "wm": wm_img.astype(f, copy=False),
        "ident": ident,
    }
    return [dict(shared, xT=xs[c].astype(f, copy=False)) for c in range(NCORES)]


def kernel(x, labels, w_gate, W1, b1, W2, b2, Wm, _trace=False):
    x = np.asarray(x, dtype=np.float32)
    in_maps = _host_prep(
        x,
        np.asarray(w_gate, np.float32),
        np.asarray(W1, np.float32),
        np.asarray(b1, np.float32),
        np.asarray(W2, np.float32),
        np.asarray(b2, np.float32),
        np.asarray(Wm, np.float32),
    )
    nc = _get_nc()
    res = bass_utils.run_bass_kernel_spmd(
        nc, in_maps, list(range(NCORES)), trace=_trace
    )
    y = np.concatenate([res.results[c]["y"] for c in range(NCORES)], axis=0)
    if _trace:
        kernel.last_results = res
    return y
